# revision 7
# baseline (speedup 1.0000x reference)
"""LMMD (DSAN local MMD) loss on 8 Trainium2 NeuronCores — triangle + fp8 V3.

Math (reference):
    X = concat(source, target)                    # [N=4096, D=1024]
    l2[i,j] = max(|x_i|^2 + |x_j|^2 - 2 x_i.x_j, 0)
    bw      = sum(l2) / (N^2 - N) / 4
    K       = sum_q exp(-l2 / (bw * 2^q)),  q = 0..4
    loss    = sum_c v_c^T K v_c / 12,  V = [s_norm; -t_norm]  (rank-12 weights)

V3 design (vs V2 triangle/fp8 baseline):
  * Bias folding: the j-side factor exp(-c_q sq_j) moves from the ACT exp
    bias into per-q host-built vt tables (vt2_q = 2 V f_q).  The es levels
    become pure powers g^(2^k) of g = exp(2 c4 G), so the group's four
    slots share ONE wide ACT exp (no per-slot bias) — one contiguous PSUM
    gram tile per group feeds it.
  * Square rebalance across engines (cost-model rates: DVE 0.52, ACT 0.83,
    Pool 1.98 ns/col): DVE squares e3/e2 + leftovers, ACT additionally
    computes e1[:, :aw] directly from the gram as exp(8s G), Pool squares
    e0[:, -pw:].  Last group skips Pool (its latency would extend the tail).
  * Scales ride in btab (bf16) — no separate ftab DMA.
  * rq is ONE PSUM tile [128, 5*4*C]; output DMA goes directly PSUM->DRAM
    (no staging copies).
"""

import numpy as np
import ml_dtypes

import concourse.bass as bass
from concourse import bacc
import concourse.mybir as mybir
import concourse.tile as tile
from concourse.bass_utils import run_bass_kernel_spmd

B = 2048
D = 1024
C = 12
NCORES = 8
N = 2 * B                 # 4096 total samples
IPC = N // NCORES         # 512 own columns (i) per core
NT = N // 128             # 32 j-tiles
NKC = D // 128            # 8 contraction chunks
NKP = NKC // 2            # 4 DoubleRow chunk-pairs
NQ = 5                    # kernels in the RBF mixture
NGROUPS = 8               # slot groups: own + 7 foreign cores
OWN_G = 0                 # own group first (cheapest start: no weight DMA)
WLAG = 3                  # groups of lag between es production and weighted use

# (block_off, block_end) in 128-col i-blocks, per slot-in-group
OWN_SPANS = [(0, 4), (1, 4), (2, 4), (3, 4)]
FOREIGN_PAT = [(0, 2), (0, 2), (2, 4), (2, 4)]

# per-group elementwise split (cols): aw = ACT's exp(8sG) share of e1,
# pw = Pool's square share at the tail of e0.
AW_FOREIGN, PW_FOREIGN = 512, 800
AW_OWN, PW_OWN = 640, 1000

# btab layout (bf16): vt2 [5*NT*C] | vt1 [5*4*C] | scales [4]
VT2_COLS = NQ * NT * C
VT1_COLS = NQ * 4 * C
BT_COLS = VT2_COLS + VT1_COLS + 4

F8NP = ml_dtypes.float8_e4m3
BFNP = ml_dtypes.bfloat16

_BUILT = None             # program is input-independent


def _slot_geom(g, sl):
    if g == OWN_G:
        off, end = OWN_SPANS[sl]
    else:
        off, end = FOREIGN_PAT[sl]
    return off, end


OWN_EBASE = [0, 640, 1024, 512]   # keeps every slot slice inside one PSUM bank


def _ebase(g, sl):
    # column base of slot sl inside the group's batched gram/e tiles.
    # A matmul output must not cross a 512-fp32 PSUM bank boundary, so the
    # own group's 512/384/256/128 spans are packed [0:512|640:1024|1024:1280|
    # 512:640] instead of cumulatively.
    if g == OWN_G:
        return OWN_EBASE[sl]
    bases = [0]
    for s in range(1, 4):
        o, e = _slot_geom(g, s - 1)
        bases.append(bases[-1] + (e - o) * 128)
    return bases[sl]


def _gsplit(g):
    gw = 1280 if g == OWN_G else 1024
    aw, pw = (AW_OWN, PW_OWN) if g == OWN_G else (AW_FOREIGN, PW_FOREIGN)
    if g == NGROUPS - 1:
        pw = 0            # keep slow Pool off the tail
    return gw, aw, pw


def _build_program():
    fp32 = mybir.dt.float32
    bf16 = mybir.dt.bfloat16
    f8 = mybir.dt.float8e4
    Exp = mybir.ActivationFunctionType.Exp
    DR = mybir.MatmulPerfMode.DoubleRow

    nc = bacc.Bacc()
    # host-pretransposed: xtb[p, k, s*128+j] = X[jseq[s]*128+j, k*128+p]
    xtb = nc.declare_dram_parameter("xtb", [128, NKC, NT * 128], f8, isOutput=False)
    btab = nc.declare_dram_parameter("btab", [128, BT_COLS], bf16, isOutput=False)
    # flipped weighted layout: rows = i within own 128-block, cols = (q, block, cls)
    rout = nc.declare_dram_parameter("r_out", [128, NQ * 4 * C], fp32, isOutput=True)

    with tile.TileContext(nc) as tc:
        with (
            tc.tile_pool(name="singles", bufs=1) as singles,
            tc.tile_pool(name="wpool", bufs=3) as wpool,
            tc.tile_pool(name="epool", bufs=5) as epool,
            tc.tile_pool(name="gpsum", bufs=2, space="PSUM") as gpsum,
            tc.tile_pool(name="rqpsum", bufs=1, space="PSUM") as rqpsum,
        ):
            # own i-columns = slots 0..3 of xtb, first on the serialized DMA
            # path (gram slot 0 starts after the first half).
            own_sb = singles.tile([128, NKC, IPC], f8)
            nc.sync.dma_start(out=own_sb[:, 0 : NKC // 2, :], in_=xtb[:, 0 : NKC // 2, 0:IPC])
            nc.sync.dma_start(out=own_sb[:, NKC // 2 : NKC, :], in_=xtb[:, NKC // 2 : NKC, 0:IPC])
            btab_sb = singles.tile([128, BT_COLS], bf16)
            # Stage small tiles through DVE so consumers wait on one
            # semaphore instead of the DMA queue fan-out (walrus caps the
            # per-instruction sync-wait count).
            vt2_s = singles.tile([128, VT2_COLS], bf16)
            vt1_s = singles.tile([128, VT1_COLS], bf16)
            # fp32 scale/bias staging: [scl_e4, scl_e1, zero, zero].  The
            # scales ride in btab's tail; fetch just those 4 cols early (the
            # bulk vt DMA would otherwise delay wg1 / the first exp).
            nc.sync.dma_start(
                out=btab_sb[:, VT2_COLS + VT1_COLS :],
                in_=btab[:, VT2_COLS + VT1_COLS :],
            )
            scl_s = singles.tile([128, 4], fp32)
            nc.vector.tensor_copy(scl_s, btab_sb[:, VT2_COLS + VT1_COLS :])
            warm = singles.tile([128, 4], fp32)
            # Dummy ACT op: loads the Exp table early and absorbs the DVE
            # wait so loop Exp ops only ever need the PE wait.
            nc.scalar.activation(warm, scl_s, Exp)

            # rq[p, q*48 + b*C + cls] accumulates R_q over j for own block b
            rq = rqpsum.tile([128, NQ * 4 * C], fp32)

            def emit_weighted(g, es):
                # Flipped orientation: es block stationary, vt moving.
                # q-major, q=4 first so PE chases the squaring chain.
                for q in range(NQ - 1, -1, -1):
                    for sl in range(4):
                        slot = 4 * g + sl
                        off, end = _slot_geom(g, sl)
                        eb = _ebase(g, sl)
                        for b in range(off, end):
                            if g == OWN_G and b == sl:
                                vtb = vt1_s[:, (q * 4 + sl) * C : (q * 4 + sl + 1) * C]
                            else:
                                vtb = vt2_s[:, (q * NT + slot) * C : (q * NT + slot + 1) * C]
                            col = eb + (b - off) * 128
                            # PSUM has_written: start clears the WHOLE bank's
                            # bits, so only the very first matmul into rq may
                            # set it — every slice then first-touch-overwrites
                            # (bit clear) and accumulates afterwards.
                            nc.tensor.matmul(
                                rq[:, q * 4 * C + b * C : q * 4 * C + (b + 1) * C],
                                lhsT=es[q][:, col : col + 128],
                                rhs=vtb,
                                start=(g == 0 and q == NQ - 1 and sl == 0 and b == 0),
                                stop=(g == NGROUPS - 1 and sl == 3 and b == 3),
                            )

            pending = []
            for g in range(NGROUPS):
                gw, aw, pw = _gsplit(g)
                if g == OWN_G:
                    wsrc = own_sb
                else:
                    wg = wpool.tile([128, NKC, 512], f8, tag="wg", name=f"w{g}")
                    src0 = g * 512
                    nc.sync.dma_start(out=wg, in_=xtb[:, :, src0 : src0 + 512])
                    if g == 1:
                        # vt tables land after wg1; first consumer is
                        # weighted(0) at ~8us so wg1 wins the DMA path
                        nc.sync.dma_start(
                            out=btab_sb[:, 0 : VT2_COLS + VT1_COLS],
                            in_=btab[:, 0 : VT2_COLS + VT1_COLS],
                        )
                        nc.vector.tensor_copy(vt2_s, btab_sb[:, 0:VT2_COLS])
                        nc.vector.tensor_copy(vt1_s, btab_sb[:, VT2_COLS : VT2_COLS + VT1_COLS])
                    wsrc = wg
                # one contiguous PSUM gram tile for the whole group
                gt = gpsum.tile([128, 1280], fp32, tag="g", name=f"g{g}")
                for sl in (range(3, -1, -1) if g == OWN_G else range(4)):
                    off, end = _slot_geom(g, sl)
                    span = (end - off) * 128
                    eb = _ebase(g, sl)
                    for m in range(NKP):
                        nc.tensor.matmul(
                            gt[:, eb : eb + span],
                            lhsT=wsrc[:, 2 * m : 2 * m + 2, sl * 128 : (sl + 1) * 128],
                            rhs=own_sb[:, 2 * m : 2 * m + 2, off * 128 : end * 128],
                            start=(m == 0),
                            stop=(m == NKP - 1),
                            perf_mode=DR,
                        )
                es = {q: epool.tile([128, 1280], bf16, tag=f"e{q}", name=f"e{q}g{g}") for q in range(NQ)}
                # e4 = exp(s*G) — one wide call, no bias (folded into vt)
                nc.scalar.activation(
                    es[4][:, 0:gw], gt[:, 0:gw], Exp,
                    bias=scl_s[:, 2:3], scale=scl_s[:, 0:1],
                )
                # ACT's share of e1 comes straight from the gram: exp(8s*G)
                nc.scalar.activation(
                    es[1][:, 0:aw], gt[:, 0:aw], Exp,
                    bias=scl_s[:, 2:3], scale=scl_s[:, 1:2],
                )
                # squaring chain on DVE (2x_1p), Pool takes the e0 tail
                nc.vector.tensor_mul(es[3][:, 0:gw], es[4][:, 0:gw], es[4][:, 0:gw])
                nc.vector.tensor_mul(es[2][:, 0:gw], es[3][:, 0:gw], es[3][:, 0:gw])
                nc.vector.tensor_mul(es[1][:, aw:gw], es[2][:, aw:gw], es[2][:, aw:gw])
                nc.vector.tensor_mul(es[0][:, 0 : gw - pw], es[1][:, 0 : gw - pw], es[1][:, 0 : gw - pw])
                if pw:
                    nc.gpsimd.tensor_mul(es[0][:, gw - pw : gw], es[1][:, gw - pw : gw], es[1][:, gw - pw : gw])
                pending.append((g, es))
                if len(pending) > WLAG:
                    emit_weighted(*pending.pop(0))
            for item in pending:
                emit_weighted(*item)

            # tail: drain each q-slice as its last matmul lands (q=4 first);
            # copies alternate DVE/ACT; single DMA after the q=0 copy.
            stg = singles.tile([128, NQ * 4 * C], fp32)
            Copy = mybir.ActivationFunctionType.Copy
            for q in range(NQ - 1, -1, -1):
                dst = stg[:, q * 4 * C : (q + 1) * 4 * C]
                src = rq[:, q * 4 * C : (q + 1) * 4 * C]
                if q % 2 == 0:
                    nc.vector.tensor_copy(dst, src)
                else:
                    nc.scalar.activation(dst, src, Copy)
            nc.sync.dma_start(out=rout[:], in_=stg)

    nc.compile()
    return nc


def _jseq(c):
    seq = list(range(4 * c, 4 * c + 4))
    for d in range(NCORES):
        if d == c:
            continue
        if d > c:
            seq += [4 * d, 4 * d + 1, 4 * d + 2, 4 * d + 3]
        else:
            seq += [4 * d + 2, 4 * d + 3, 4 * d, 4 * d + 1]
    return seq


def _prep(source, target, source_label, target_logits):
    X = np.concatenate([np.asarray(source), np.asarray(target)], axis=0)
    X64 = X.astype(np.float64)
    sq = np.einsum("nd,nd->n", X64, X64)
    colsum = X64.sum(axis=0)
    sum_l2 = 2.0 * N * sq.sum() - 2.0 * (colsum @ colsum)
    bw = sum_l2 / (N * N - N) / (2.0 ** (NQ // 2))
    cq = np.array([1.0 / (bw * 2.0**q) for q in range(NQ)])  # [5]

    sl = np.asarray(source_label, np.float64)
    tl = np.asarray(target_logits, np.float64)
    ssum = sl.sum(0)
    s_norm = np.where(ssum > 0, sl / np.where(ssum > 0, ssum, 1.0), 0.0)
    tsum = tl.sum(0)
    t_norm = np.where(tsum > 0, tl / np.where(tsum > 0, tsum, 1.0), 0.0)
    s_pres = np.zeros(C)
    np.add.at(s_pres, sl.argmax(1), 1.0)
    t_pres = np.zeros(C)
    np.add.at(t_pres, tl.argmax(1), 1.0)
    common = ((s_pres > 0) & (t_pres > 0)).astype(np.float64)
    V = np.concatenate([s_norm * common, -t_norm * common], axis=0)  # [N, C]

    # j-side RBF bias folded into the vt tables: vt2_q = 2 V f_q
    fq = np.exp(-np.outer(cq, sq))                        # [5, N]

    # fp8 X^T in [p, k, jcol] layout (global j order; per-core slot perm later)
    X8 = X.astype(F8NP)                                   # [N, D]
    xt8 = np.ascontiguousarray(
        X8.T.reshape(NKC, 128, N).transpose(1, 0, 2)      # [128, 8, N]
    )
    return X, sq, cq, V, fq, xt8


def _core_inputs(c, cq, V, fq, xt8):
    seq = _jseq(c)
    # xtb: permute j-tiles into slot order
    xtb = np.ascontiguousarray(
        xt8.reshape(128, NKC, NT, 128)[:, :, seq, :].reshape(128, NKC, NT * 128)
    )
    # Vq[q, j, cls] = V[j, cls] * f_q[j], j in slot order
    Vt = V.reshape(NT, 128, C)[seq]                       # [NT, 128, C]
    fqt = fq.reshape(NQ, NT, 128)[:, seq]                 # [NQ, NT, 128]
    Vq = Vt[None] * fqt[..., None]                        # [NQ, NT, 128, C]
    vt2 = (2.0 * Vq).transpose(2, 0, 1, 3).reshape(128, NQ * NT * C)
    vt1 = Vq[:, :4].transpose(2, 0, 1, 3).reshape(128, NQ * 4 * C)
    scl = np.zeros((128, 4))
    scl[:, 0] = 2.0 * cq[4]
    scl[:, 1] = 2.0 * cq[1]
    btab = np.ascontiguousarray(
        np.concatenate([vt2, vt1, scl], axis=1)
    ).astype(BFNP)
    return {"xtb": xtb, "btab": btab}


def _postprocess(results, sq, cq, V):
    # loss = 1/12 sum_q sum_i alpha_q[i] * (sum_cls V[i,cls] R_q[cls,i])
    loss = 0.0
    for c in range(NCORES):
        # r[p, q, b, cls] = R_q[cls, i] at i = 512c + 128b + p
        r = np.asarray(results[c]["r_out"], np.float64).reshape(128, NQ, 4, C)
        gi = c * IPC + np.arange(IPC)
        Vc = V[gi].reshape(4, 128, C)                     # [b, p, cls]
        alpha = np.exp(-np.outer(cq, sq[gi])).reshape(NQ, 4, 128)
        loss += np.einsum("qbp,bpc,pqbc->", alpha, Vc, r)
    return loss / C


def _run(in_maps, trace=False, **kw):
    global _BUILT
    if _BUILT is None:
        _BUILT = _build_program()
    return run_bass_kernel_spmd(_BUILT, in_maps, list(range(NCORES)), trace=trace, **kw)


def kernel(source, target, source_label, target_logits, _trace=False, _ret_bkr=False):
    X, sq, cq, V, fq, xt8 = _prep(source, target, source_label, target_logits)
    in_maps = [_core_inputs(c, cq, V, fq, xt8) for c in range(NCORES)]
    bkr = None
    for attempt in range(3):
        try:
            bkr = _run(in_maps, trace=_trace)
            break
        except Exception:
            # transient device wedge (NRT_EXEC_UNIT_UNRECOVERABLE) — back off
            # briefly and retry; the device recovers on a fresh session
            if attempt == 2:
                raise
            import time as _time

            _time.sleep(2.0)
    loss = _postprocess(bkr.results, sq, cq, V)
    out = np.float32(loss)
    if _ret_bkr:
        return out, bkr
    return out


# revision 10
# speedup vs baseline: 1.0609x; 1.0609x over previous
"""LMMD (DSAN local MMD) loss on 8 Trainium2 NeuronCores — triangle + fp8 V3.

Math (reference):
    X = concat(source, target)                    # [N=4096, D=1024]
    l2[i,j] = max(|x_i|^2 + |x_j|^2 - 2 x_i.x_j, 0)
    bw      = sum(l2) / (N^2 - N) / 4
    K       = sum_q exp(-l2 / (bw * 2^q)),  q = 0..4
    loss    = sum_c v_c^T K v_c / 12,  V = [s_norm; -t_norm]  (rank-12 weights)

V3 design (vs V2 triangle/fp8 baseline):
  * Bias folding: the j-side factor exp(-c_q sq_j) moves from the ACT exp
    bias into per-q host-built vt tables (vt2_q = 2 V f_q).  The es levels
    become pure powers g^(2^k) of g = exp(2 c4 G), so the group's four
    slots share ONE wide ACT exp (no per-slot bias) — one contiguous PSUM
    gram tile per group feeds it.
  * Square rebalance across engines (cost-model rates: DVE 0.52, ACT 0.83,
    Pool 1.98 ns/col): DVE squares e3/e2 + leftovers, ACT additionally
    computes e1[:, :aw] directly from the gram as exp(8s G), Pool squares
    e0[:, -pw:].  Last group skips Pool (its latency would extend the tail).
  * Scales ride in btab (bf16) — no separate ftab DMA.
  * rq is ONE PSUM tile [128, 5*4*C]; output DMA goes directly PSUM->DRAM
    (no staging copies).
"""

import numpy as np
import ml_dtypes

import concourse.bass as bass
from concourse import bacc
import concourse.mybir as mybir
import concourse.tile as tile
from concourse.bass_utils import run_bass_kernel_spmd

B = 2048
D = 1024
C = 12
NCORES = 8
N = 2 * B                 # 4096 total samples
IPC = N // NCORES         # 512 own columns (i) per core
NT = N // 128             # 32 j-tiles
NKC = D // 128            # 8 contraction chunks
NKP = NKC // 2            # 4 DoubleRow chunk-pairs
NQ = 5                    # kernels in the RBF mixture
NGROUPS = 8               # slot groups: own + 7 foreign cores
OWN_G = 0                 # own group first (cheapest start: no weight DMA)
WLAG = 3                  # groups of lag between es production and weighted use

# (block_off, block_end) in 128-col i-blocks, per slot-in-group
OWN_SPANS = [(0, 4), (1, 4), (2, 4), (3, 4)]
FOREIGN_PAT = [(0, 2), (0, 2), (2, 4), (2, 4)]

# per-group elementwise split (cols): aw = ACT's exp(8sG) share of e1,
# pw = Pool's square share at the tail of e0.
AW_FOREIGN, PW_FOREIGN = 512, 800
AW_OWN, PW_OWN = 640, 1000

# btab layout (bf16): vt2 [5*NT*C] | vt1 [5*4*C] | scales [4]
VT2_COLS = NQ * NT * C
VT1_COLS = NQ * 4 * C
BT_COLS = VT2_COLS + VT1_COLS + 4

F8NP = ml_dtypes.float8_e4m3
BFNP = ml_dtypes.bfloat16

_BUILT = None             # program is input-independent


def _slot_geom(g, sl):
    if g == OWN_G:
        off, end = OWN_SPANS[sl]
    else:
        off, end = FOREIGN_PAT[sl]
    return off, end


OWN_EBASE = [0, 640, 1024, 512]   # keeps every slot slice inside one PSUM bank


def _ebase(g, sl):
    # column base of slot sl inside the group's batched gram/e tiles.
    # A matmul output must not cross a 512-fp32 PSUM bank boundary, so the
    # own group's 512/384/256/128 spans are packed [0:512|640:1024|1024:1280|
    # 512:640] instead of cumulatively.
    if g == OWN_G:
        return OWN_EBASE[sl]
    bases = [0]
    for s in range(1, 4):
        o, e = _slot_geom(g, s - 1)
        bases.append(bases[-1] + (e - o) * 128)
    return bases[sl]


def _gsplit(g):
    gw = 1280 if g == OWN_G else 1024
    aw, pw = (AW_OWN, PW_OWN) if g == OWN_G else (AW_FOREIGN, PW_FOREIGN)
    if g == NGROUPS - 1:
        pw = 0            # keep slow Pool off the tail
    return gw, aw, pw


def _build_program():
    fp32 = mybir.dt.float32
    bf16 = mybir.dt.bfloat16
    f8 = mybir.dt.float8e4
    Exp = mybir.ActivationFunctionType.Exp
    DR = mybir.MatmulPerfMode.DoubleRow

    nc = bacc.Bacc()
    # host-pretransposed: xtb[p, k, s*128+j] = X[jseq[s]*128+j, k*128+p]
    xtb = nc.declare_dram_parameter("xtb", [128, NKC, NT * 128], f8, isOutput=False)
    btab = nc.declare_dram_parameter("btab", [128, BT_COLS], bf16, isOutput=False)
    # flipped weighted layout: rows = i within own 128-block, cols = (q, block, cls)
    rout = nc.declare_dram_parameter("r_out", [128, NQ * 4 * C], fp32, isOutput=True)

    with tile.TileContext(nc) as tc:
        with (
            tc.tile_pool(name="singles", bufs=1) as singles,
            tc.tile_pool(name="wpool", bufs=3) as wpool,
            tc.tile_pool(name="epool", bufs=5) as epool,
            tc.tile_pool(name="gpsum", bufs=2, space="PSUM") as gpsum,
            tc.tile_pool(name="rqpsum", bufs=1, space="PSUM") as rqpsum,
        ):
            # own i-columns = slots 0..3 of xtb, first on the serialized DMA
            # path (gram slot 0 starts after the first half).
            own_sb = singles.tile([128, NKC, IPC], f8)
            nc.sync.dma_start(out=own_sb[:, 0 : NKC // 2, :], in_=xtb[:, 0 : NKC // 2, 0:IPC])
            nc.sync.dma_start(out=own_sb[:, NKC // 2 : NKC, :], in_=xtb[:, NKC // 2 : NKC, 0:IPC])
            btab_sb = singles.tile([128, BT_COLS], bf16)
            # Stage small tiles through DVE so consumers wait on one
            # semaphore instead of the DMA queue fan-out (walrus caps the
            # per-instruction sync-wait count).
            vt2_s = singles.tile([128, VT2_COLS], bf16)
            vt1_s = singles.tile([128, VT1_COLS], bf16)
            # fp32 scale/bias staging: [scl_e4, scl_e1, zero, zero].  The
            # scales ride in btab's tail; fetch just those 4 cols early (the
            # bulk vt DMA would otherwise delay wg1 / the first exp).
            nc.sync.dma_start(
                out=btab_sb[:, VT2_COLS + VT1_COLS :],
                in_=btab[:, VT2_COLS + VT1_COLS :],
            )
            scl_s = singles.tile([128, 4], fp32)
            nc.vector.tensor_copy(scl_s, btab_sb[:, VT2_COLS + VT1_COLS :])
            warm = singles.tile([128, 4], fp32)
            # Dummy ACT op: loads the Exp table early and absorbs the DVE
            # wait so loop Exp ops only ever need the PE wait.
            nc.scalar.activation(warm, scl_s, Exp)

            # rq[p, q*48 + b*C + cls] accumulates R_q over j for own block b.
            # Two tiles (q>=2 | q<=1) so the high-q drain copies overlap the
            # remaining low-q weighted matmuls; each tile = one PSUM bank.
            rq_hi = rqpsum.tile([128, 3 * 4 * C], fp32, tag="rqh", name="rq_hi")
            rq_lo = rqpsum.tile([128, 2 * 4 * C], fp32, tag="rql", name="rq_lo")

            def rq_slice(q, b):
                if q >= 2:
                    return rq_hi[:, (q - 2) * 4 * C + b * C : (q - 2) * 4 * C + (b + 1) * C]
                return rq_lo[:, q * 4 * C + b * C : q * 4 * C + (b + 1) * C]

            def emit_weighted(g, es):
                # Flipped orientation: es block stationary, vt moving.
                # q-major, q=4 first so PE chases the squaring chain.
                for q in range(NQ - 1, -1, -1):
                    for sl in range(4):
                        slot = 4 * g + sl
                        off, end = _slot_geom(g, sl)
                        eb = _ebase(g, sl)
                        for b in range(off, end):
                            if g == OWN_G and b == sl:
                                vtb = vt1_s[:, (q * 4 + sl) * C : (q * 4 + sl + 1) * C]
                            else:
                                vtb = vt2_s[:, (q * NT + slot) * C : (q * NT + slot + 1) * C]
                            col = eb + (b - off) * 128
                            # PSUM has_written: start clears the WHOLE bank's
                            # bits, so only the first matmul into each rq bank
                            # may set it — every slice then first-touch-
                            # overwrites (bit clear) and accumulates after.
                            nc.tensor.matmul(
                                rq_slice(q, b),
                                lhsT=es[q][:, col : col + 128],
                                rhs=vtb,
                                start=(g == 0 and q in (NQ - 1, 1) and sl == 0 and b == 0),
                                stop=(g == NGROUPS - 1 and q in (2, 0) and sl == 3 and b == 3),
                            )

            pending = []
            for g in range(NGROUPS):
                gw, aw, pw = _gsplit(g)
                if g == OWN_G:
                    wsrc = own_sb
                else:
                    wg = wpool.tile([128, NKC, 512], f8, tag="wg", name=f"w{g}")
                    src0 = g * 512
                    nc.sync.dma_start(out=wg, in_=xtb[:, :, src0 : src0 + 512])
                    if g == 1:
                        # vt tables land after wg1; first consumer is
                        # weighted(0) at ~8us so wg1 wins the DMA path
                        nc.sync.dma_start(
                            out=btab_sb[:, 0 : VT2_COLS + VT1_COLS],
                            in_=btab[:, 0 : VT2_COLS + VT1_COLS],
                        )
                        nc.vector.tensor_copy(vt2_s, btab_sb[:, 0:VT2_COLS])
                        nc.vector.tensor_copy(vt1_s, btab_sb[:, VT2_COLS : VT2_COLS + VT1_COLS])
                    wsrc = wg
                # one contiguous PSUM gram tile for the whole group
                gt = gpsum.tile([128, 1280], fp32, tag="g", name=f"g{g}")
                for sl in (range(3, -1, -1) if g == OWN_G else range(4)):
                    off, end = _slot_geom(g, sl)
                    span = (end - off) * 128
                    eb = _ebase(g, sl)
                    for m in range(NKP):
                        nc.tensor.matmul(
                            gt[:, eb : eb + span],
                            lhsT=wsrc[:, 2 * m : 2 * m + 2, sl * 128 : (sl + 1) * 128],
                            rhs=own_sb[:, 2 * m : 2 * m + 2, off * 128 : end * 128],
                            start=(m == 0),
                            stop=(m == NKP - 1),
                            perf_mode=DR,
                        )
                es = {q: epool.tile([128, 1280], bf16, tag=f"e{q}", name=f"e{q}g{g}") for q in range(NQ)}
                # e4 = exp(s*G) — one wide call, no bias (folded into vt)
                nc.scalar.activation(
                    es[4][:, 0:gw], gt[:, 0:gw], Exp,
                    bias=scl_s[:, 2:3], scale=scl_s[:, 0:1],
                )
                # ACT's share of e1 comes straight from the gram: exp(8s*G)
                nc.scalar.activation(
                    es[1][:, 0:aw], gt[:, 0:aw], Exp,
                    bias=scl_s[:, 2:3], scale=scl_s[:, 1:2],
                )
                # squaring chain on DVE (2x_1p), Pool takes the e0 tail
                nc.vector.tensor_mul(es[3][:, 0:gw], es[4][:, 0:gw], es[4][:, 0:gw])
                nc.vector.tensor_mul(es[2][:, 0:gw], es[3][:, 0:gw], es[3][:, 0:gw])
                nc.vector.tensor_mul(es[1][:, aw:gw], es[2][:, aw:gw], es[2][:, aw:gw])
                nc.vector.tensor_mul(es[0][:, 0 : gw - pw], es[1][:, 0 : gw - pw], es[1][:, 0 : gw - pw])
                if pw:
                    nc.gpsimd.tensor_mul(es[0][:, gw - pw : gw], es[1][:, gw - pw : gw], es[1][:, gw - pw : gw])
                pending.append((g, es))
                if len(pending) > WLAG:
                    emit_weighted(*pending.pop(0))
            for item in pending:
                emit_weighted(*item)

            # tail: the q>=2 tile drains (ACT) while q1/q0 matmuls still run,
            # then one DVE copy for the low tile and a single DMA.
            stg = singles.tile([128, NQ * 4 * C], fp32)
            Copy = mybir.ActivationFunctionType.Copy
            nc.scalar.activation(stg[:, 2 * 4 * C :], rq_hi, Copy)
            nc.vector.tensor_copy(stg[:, 0 : 2 * 4 * C], rq_lo)
            nc.sync.dma_start(out=rout[:], in_=stg)

    nc.compile()
    return nc


def _jseq(c):
    seq = list(range(4 * c, 4 * c + 4))
    for d in range(NCORES):
        if d == c:
            continue
        if d > c:
            seq += [4 * d, 4 * d + 1, 4 * d + 2, 4 * d + 3]
        else:
            seq += [4 * d + 2, 4 * d + 3, 4 * d, 4 * d + 1]
    return seq


def _prep(source, target, source_label, target_logits):
    X = np.concatenate([np.asarray(source), np.asarray(target)], axis=0)
    X64 = X.astype(np.float64)
    sq = np.einsum("nd,nd->n", X64, X64)
    colsum = X64.sum(axis=0)
    sum_l2 = 2.0 * N * sq.sum() - 2.0 * (colsum @ colsum)
    bw = sum_l2 / (N * N - N) / (2.0 ** (NQ // 2))
    cq = np.array([1.0 / (bw * 2.0**q) for q in range(NQ)])  # [5]

    sl = np.asarray(source_label, np.float64)
    tl = np.asarray(target_logits, np.float64)
    ssum = sl.sum(0)
    s_norm = np.where(ssum > 0, sl / np.where(ssum > 0, ssum, 1.0), 0.0)
    tsum = tl.sum(0)
    t_norm = np.where(tsum > 0, tl / np.where(tsum > 0, tsum, 1.0), 0.0)
    s_pres = np.zeros(C)
    np.add.at(s_pres, sl.argmax(1), 1.0)
    t_pres = np.zeros(C)
    np.add.at(t_pres, tl.argmax(1), 1.0)
    common = ((s_pres > 0) & (t_pres > 0)).astype(np.float64)
    V = np.concatenate([s_norm * common, -t_norm * common], axis=0)  # [N, C]

    # j-side RBF bias folded into the vt tables: vt2_q = 2 V f_q
    fq = np.exp(-np.outer(cq, sq))                        # [5, N]

    # fp8 X^T in [p, k, jcol] layout (global j order; per-core slot perm later)
    X8 = X.astype(F8NP)                                   # [N, D]
    xt8 = np.ascontiguousarray(
        X8.T.reshape(NKC, 128, N).transpose(1, 0, 2)      # [128, 8, N]
    )
    return X, sq, cq, V, fq, xt8


def _core_inputs(c, cq, V, fq, xt8):
    seq = _jseq(c)
    # xtb: permute j-tiles into slot order
    xtb = np.ascontiguousarray(
        xt8.reshape(128, NKC, NT, 128)[:, :, seq, :].reshape(128, NKC, NT * 128)
    )
    # Vq[q, j, cls] = V[j, cls] * f_q[j], j in slot order
    Vt = V.reshape(NT, 128, C)[seq]                       # [NT, 128, C]
    fqt = fq.reshape(NQ, NT, 128)[:, seq]                 # [NQ, NT, 128]
    Vq = Vt[None] * fqt[..., None]                        # [NQ, NT, 128, C]
    vt2 = (2.0 * Vq).transpose(2, 0, 1, 3).reshape(128, NQ * NT * C)
    vt1 = Vq[:, :4].transpose(2, 0, 1, 3).reshape(128, NQ * 4 * C)
    scl = np.zeros((128, 4))
    scl[:, 0] = 2.0 * cq[4]
    scl[:, 1] = 2.0 * cq[1]
    btab = np.ascontiguousarray(
        np.concatenate([vt2, vt1, scl], axis=1)
    ).astype(BFNP)
    return {"xtb": xtb, "btab": btab}


def _postprocess(results, sq, cq, V):
    # loss = 1/12 sum_q sum_i alpha_q[i] * (sum_cls V[i,cls] R_q[cls,i])
    loss = 0.0
    for c in range(NCORES):
        # r[p, q, b, cls] = R_q[cls, i] at i = 512c + 128b + p
        r = np.asarray(results[c]["r_out"], np.float64).reshape(128, NQ, 4, C)
        gi = c * IPC + np.arange(IPC)
        Vc = V[gi].reshape(4, 128, C)                     # [b, p, cls]
        alpha = np.exp(-np.outer(cq, sq[gi])).reshape(NQ, 4, 128)
        loss += np.einsum("qbp,bpc,pqbc->", alpha, Vc, r)
    return loss / C


def _run(in_maps, trace=False, **kw):
    global _BUILT
    if _BUILT is None:
        _BUILT = _build_program()
    return run_bass_kernel_spmd(_BUILT, in_maps, list(range(NCORES)), trace=trace, **kw)


def kernel(source, target, source_label, target_logits, _trace=False, _ret_bkr=False):
    X, sq, cq, V, fq, xt8 = _prep(source, target, source_label, target_logits)
    in_maps = [_core_inputs(c, cq, V, fq, xt8) for c in range(NCORES)]
    bkr = None
    for attempt in range(3):
        try:
            bkr = _run(in_maps, trace=_trace)
            break
        except Exception:
            # transient device wedge (NRT_EXEC_UNIT_UNRECOVERABLE) — back off
            # briefly and retry; the device recovers on a fresh session
            if attempt == 2:
                raise
            import time as _time

            _time.sleep(2.0)
    loss = _postprocess(bkr.results, sq, cq, V)
    out = np.float32(loss)
    if _ret_bkr:
        return out, bkr
    return out
